# revision 1
# baseline (speedup 1.0000x reference)
"""Multi-head attention (B=2, N=2048, C=768, H=12) on 8 trn2 cores.

Sharding: core i handles batch b = i//4 and head-group g = i%4 (3 heads each).
Per-core pipeline:
  1. QKV^T projection from host-pre-transposed xT [C, N] (fp32r matmuls):
       qT, kT produced d-major [64, N] per head; v produced n-major [N, 64]
       per head, augmented with a ones column (softmax denominator trick).
  2. Scores computed transposed: S^T[k, q] = kT_h.T-slice @ qT_h, so softmax
       (exp via ScalarE) and attn@V need no on-chip transposes.
  3. attn@V with lhsT = [v | 1]: out rows 0:64 = unnormalized attn_out^T,
       row 64 = softmax denominators.
  4. Normalize (fp32): reciprocal of row 64, PE-broadcast to 64 partitions,
       DVE multiply.
  5. AllGather attn_out^T shards [192, N] -> [768, N] within groups
       [[0,1,2,3],[4,5,6,7]] (same batch).
  6. Output projection column-sharded by w_proj columns (per-core input
       shard), bias folded in as a K=1 matmul. Output is out^T [192, N];
       host concatenates + transposes.

Matmuls run in float32r (1 PE cycle/row vs 4 for fp32); the softmax
normalization chain stays fp32.
"""

import numpy as np

B, N, C, H, HD = 2, 2048, 768, 12, 64
G = 4              # tensor-parallel head groups
HL = H // G        # 3 heads per core
CHL = HL * HD      # 192 local channels
SCALE = HD ** -0.5
NCORES = 8
CT = C // 128      # 6 contraction chunks
NT = N // 128      # 16 n chunks
QW = 1024          # q window width
NWIN = N // QW     # 2 windows
KT = N // 128      # 16 k chunks
FW = 512           # matmul free width (psum bank)

_CACHE = {}


def _build_nc():
    import os
    import concourse.bass as bass
    import concourse.bacc as bacc
    import concourse.tile as tile
    import concourse.mybir as mybir

    F32 = mybir.dt.float32
    F32R = mybir.dt.float32r
    AF = mybir.ActivationFunctionType
    debug = bool(int(os.environ.get("KERNEL_DEBUG", "0")))

    nc = bacc.Bacc(num_devices=NCORES)
    xT_d = nc.declare_dram_parameter("xT", [C, N], F32R, isOutput=False)
    wq_d = nc.declare_dram_parameter("wq", [C, CHL], F32R, isOutput=False)
    wk_d = nc.declare_dram_parameter("wk", [C, CHL], F32R, isOutput=False)
    wv_d = nc.declare_dram_parameter("wv", [C, CHL], F32R, isOutput=False)
    wpz_d = nc.declare_dram_parameter("wpz", [NCORES * CHL, CHL], F32R,
                                      isOutput=False)
    bp_d = nc.declare_dram_parameter("bp", [1, CHL], F32R, isOutput=False)
    out_d = nc.declare_dram_parameter("out", [CHL, N], F32, isOutput=True)
    if debug:
        dbg_sums = nc.declare_dram_parameter("dbg_sums", [HL * NWIN, QW], F32,
                                             isOutput=True)
        dbg_recip = nc.declare_dram_parameter("dbg_recip", [HL * NWIN, QW], F32,
                                              isOutput=True)

    with tile.TileContext(nc) as tc:
        with tc.tile_pool(name="dram", bufs=1, space="DRAM") as dram:
            ag_ins = [dram.tile([CHL, QW], F32R, name=f"ag_in{w}")
                      for w in range(NWIN)]
            ag_outs = [dram.tile([NCORES * CHL, QW], F32R, name=f"ag_out{w}",
                                 addr_space="Shared")
                       for w in range(NWIN)]

            with tc.tile_pool(name="persist", bufs=1) as P:
                # ---- inputs only needed through phase 1 (own pool) ----
                QIN = tc.alloc_tile_pool(name="qkv_in", bufs=1)
                xT_sb = QIN.tile([128, CT, N], F32R)
                for ct in range(CT):
                    nc.sync.dma_start(
                        out=xT_sb[:, ct, :],
                        in_=xT_d[ct * 128:(ct + 1) * 128, :],
                    )
                wq_sb = QIN.tile([128, CT, CHL], F32R)
                wk_sb = QIN.tile([128, CT, CHL], F32R)
                wv_sb = QIN.tile([128, CT, CHL], F32R)
                for wsb, wd in ((wq_sb, wq_d), (wk_sb, wk_d),
                                (wv_sb, wv_d)):
                    for ct in range(CT):
                        nc.sync.dma_start(
                            out=wsb[:, ct, :],
                            in_=wd[ct * 128:(ct + 1) * 128, :],
                        )
                KP = NCORES * CHL // 128   # 12 K-chunks over gathered rows
                wp_sb = P.tile([128, KP, CHL], F32R)
                for kp in range(KP):
                    nc.sync.dma_start(
                        out=wp_sb[:, kp, :],
                        in_=wpz_d[kp * 128:(kp + 1) * 128, :],
                    )
                bp_sb = P.tile([1, CHL], F32R)
                nc.sync.dma_start(out=bp_sb[:], in_=bp_d[:, :])

                ones_sb = P.tile([128, 64], F32)
                nc.vector.memset(ones_sb[:], 1.0)
                ones_q = P.tile([1, FW], F32R)
                nc.vector.memset(ones_q[:].bitcast(F32), 1.0)

                # ---- persistent QKV results ----
                q01_sb = P.tile([128, N], F32R)   # qT heads 0,1
                q2_sb = P.tile([64, N], F32R)     # qT head 2
                k01_sb = P.tile([128, N], F32R)
                k2_sb = P.tile([64, N], F32R)
                # [n, nt, h, 128]: col 0 = ones (sums), 1:64 zero, 64:128 = v
                v_sb = P.tile([128, NT, HL, 2 * HD], F32R)
                nc.vector.memset(v_sb[:, :, :, 0:1].bitcast(F32), 1.0)
                nc.vector.memset(v_sb[:, :, :, 1:HD].bitcast(F32), 0.0)

                # ---- phase 1: QKV projections ----
                with tc.tile_pool(name="qkv_ps", bufs=1, space="PSUM") as QP:
                    for dst, wsb, mlo, mhi in (
                        (q01_sb, wq_sb, 0, 128),
                        (q2_sb, wq_sb, 128, CHL),
                        (k01_sb, wk_sb, 0, 128),
                        (k2_sb, wk_sb, 128, CHL),
                    ):
                        m = mhi - mlo
                        for f in range(N // FW):
                            qk_ps = QP.tile([m, FW], F32, tag="qk", bufs=3,
                                            padded_shape=[128, FW])
                            for ct in range(CT):
                                nc.tensor.matmul(
                                    qk_ps[:],
                                    lhsT=wsb[:, ct, mlo:mhi],
                                    rhs=xT_sb[:, ct, f * FW:(f + 1) * FW],
                                    start=(ct == 0), stop=(ct == CT - 1),
                                )
                            nc.vector.tensor_copy(
                                dst[:, f * FW:(f + 1) * FW], qk_ps[:])
                    for nt in range(NT):
                        v_ps = QP.tile([128, CHL], F32, tag="v", bufs=2)
                        for ct in range(CT):
                            nc.tensor.matmul(
                                v_ps[:],
                                lhsT=xT_sb[:, ct, nt * 128:(nt + 1) * 128],
                                rhs=wv_sb[:, ct, :],
                                start=(ct == 0), stop=(ct == CT - 1),
                            )
                        nc.vector.tensor_copy(
                            v_sb[:, nt, :, HD:2 * HD],
                            v_ps[:].rearrange("p (h d) -> p h d", h=HL))
                QIN.release()

                # ---- phase 2: attention per (head, q-window) ----
                with tc.tile_pool(name="att_ps", bufs=1, space="PSUM") as AT, \
                        tc.tile_pool(name="att_sb", bufs=1) as AS:
                    for w in range(NWIN):
                        for h in range(HL):
                            qh = (q01_sb[0:64], q01_sb[64:128], q2_sb[0:64])[h]
                            kh = (k01_sb[0:64], k01_sb[64:128], k2_sb[0:64])[h]
                            q0 = w * QW
                            A = AT.tile([128, QW], F32, tag="A", bufs=1)
                            for kc in range(KT):
                                S = AT.tile([128, QW], F32, tag="S", bufs=3)
                                E = AS.tile([128, QW], F32R, tag="E", bufs=4)
                                for j in range(QW // FW):
                                    nc.tensor.matmul(
                                        S[:, j * FW:(j + 1) * FW],
                                        lhsT=kh[:, kc * 128:(kc + 1) * 128],
                                        rhs=qh[:, q0 + j * FW:q0 + (j + 1) * FW],
                                    )
                                nc.scalar.activation(E[:], S[:], AF.Exp,
                                                     scale=SCALE)
                                for j in range(QW // FW):
                                    nc.tensor.matmul(
                                        A[:, j * FW:(j + 1) * FW],
                                        lhsT=v_sb[:, kc, h, :],
                                        rhs=E[:, j * FW:(j + 1) * FW],
                                        start=(kc == 0), stop=(kc == KT - 1),
                                    )
                            # normalize: recip of row 0 (denominators),
                            # gpsimd-broadcast to all partitions, multiply.
                            R = AS.tile([1, QW], F32, tag="R", bufs=2)
                            bcs = AS.tile([128, QW], F32, tag="bcs", bufs=2)
                            attn_t = AS.tile([128, QW], F32R, tag="attn",
                                             bufs=3)
                            for j in range(QW // FW):
                                js = slice(j * FW, (j + 1) * FW)
                                nc.vector.reciprocal(R[0:1, js], A[0:1, js])
                                nc.gpsimd.partition_broadcast(
                                    bcs[:, js], R[0:1, js])
                                nc.vector.tensor_mul(attn_t[64:128, js],
                                                     A[64:128, js],
                                                     bcs[64:128, js])
                            nc.sync.dma_start(
                                out=ag_ins[w][h * HD:(h + 1) * HD, :],
                                in_=attn_t[64:128, :],
                            )
                            if debug:
                                dsum = AS.tile([65, QW], F32, tag="dsum",
                                               bufs=2)
                                nc.vector.tensor_copy(dsum[0:1, :],
                                                      A[0:1, :])
                                nc.sync.dma_start(
                                    out=dbg_sums[h * NWIN + w:h * NWIN + w + 1, :],
                                    in_=dsum[0:1, :])
                                nc.sync.dma_start(
                                    out=dbg_recip[h * NWIN + w:h * NWIN + w + 1, :],
                                    in_=R[0:1, :])
                        # per-window 8-core AllGather: window 0's gather
                        # overlaps window 1's attention compute
                        nc.gpsimd.collective_compute(
                            "AllGather",
                            mybir.AluOpType.bypass,
                            replica_groups=[list(range(NCORES))],
                            ins=[ag_ins[w].opt()],
                            outs=[ag_outs[w].opt()],
                        )

                # ---- phase 4: output projection (out^T [CHL, N]) ----
                with tc.tile_pool(name="proj_ps", bufs=1, space="PSUM") as PP, \
                        tc.tile_pool(name="proj_sb", bufs=1) as PS:
                    for f in range(N // FW):
                        wf, jf = divmod(f, QW // FW)
                        ao_ts = []
                        for kp in range(KP):
                            ao_t = PS.tile([128, FW], F32R, tag="ao",
                                           bufs=2 * KP)
                            nc.sync.dma_start(
                                out=ao_t[:],
                                in_=ag_outs[wf][kp * 128:(kp + 1) * 128,
                                                jf * FW:(jf + 1) * FW],
                            )
                            ao_ts.append(ao_t)
                        for mlo, mhi in ((0, 128), (128, CHL)):
                            m = mhi - mlo
                            pr_ps = PP.tile([m, FW], F32, tag="pr", bufs=4,
                                            padded_shape=[128, FW])
                            first = True
                            for kp in range(KP):
                                nc.tensor.matmul(
                                    pr_ps[:],
                                    lhsT=wp_sb[:, kp, mlo:mhi],
                                    rhs=ao_ts[kp][:],
                                    start=first, stop=False,
                                )
                                first = False
                            nc.tensor.matmul(
                                pr_ps[:],
                                lhsT=bp_sb[:, mlo:mhi],
                                rhs=ones_q[:],
                                start=False, stop=True,
                            )
                            o_t = PS.tile([m, FW], F32, tag="o", bufs=3,
                                          padded_shape=[128, FW])
                            nc.vector.tensor_copy(o_t[:], pr_ps[:])
                            nc.sync.dma_start(
                                out=out_d[mlo:mhi, f * FW:(f + 1) * FW],
                                in_=o_t[:],
                            )
    nc.finalize()
    return nc


def get_nc():
    if "nc" not in _CACHE:
        _CACHE["nc"] = _build_nc()
    return _CACHE["nc"]


def make_in_maps(x, w_qkv, w_proj, b_proj):
    x = np.asarray(x, dtype=np.float32)
    w_qkv = np.asarray(w_qkv, dtype=np.float32)
    w_proj = np.asarray(w_proj, dtype=np.float32)
    b_proj = np.asarray(b_proj, dtype=np.float32)
    in_maps = []
    for core in range(NCORES):
        b, g = divmod(core, G)
        cs = slice(g * CHL, (g + 1) * CHL)
        im = {
            "xT": np.ascontiguousarray(x[b].T),
            "wq": np.ascontiguousarray(w_qkv[:, 0 * C:1 * C][:, cs]),
            "wk": np.ascontiguousarray(w_qkv[:, 1 * C:2 * C][:, cs]),
            "wv": np.ascontiguousarray(w_qkv[:, 2 * C:3 * C][:, cs]),
            "bp": np.ascontiguousarray(b_proj[cs].reshape(1, CHL)),
        }
        wpz = np.zeros((NCORES * CHL, CHL), np.float32)
        for j in range(NCORES):
            if j // G == b:
                gj = j % G
                wpz[j * CHL:(j + 1) * CHL] = \
                    w_proj[gj * CHL:(gj + 1) * CHL, cs]
        im["wpz"] = wpz
        in_maps.append(im)
    return in_maps


def unshard(results):
    out = np.empty((B, N, C), dtype=np.float32)
    for b in range(B):
        outT = np.concatenate(
            [results[b * G + g]["out"] for g in range(G)], axis=0)
        out[b] = outT.T
    return out


def kernel(x, w_qkv, w_proj, b_proj):
    from concourse.bass_utils import run_bass_kernel_spmd

    nc = get_nc()
    in_maps = make_in_maps(x, w_qkv, w_proj, b_proj)
    res = run_bass_kernel_spmd(nc, in_maps, list(range(NCORES)))
    return unshard(res.results)



# revision 6
# speedup vs baseline: 1.2051x; 1.2051x over previous
"""Multi-head attention (B=2, N=2048, C=768, H=12) on 8 trn2 cores.

Sharding: core i handles batch b = i//4 and head-group g = i%4 (3 heads).
All device data is fp16 (tolerance 2e-2 allows it); matmul accumulation
stays fp32 in PSUM.

Per-core pipeline:
  1. QKV^T projection from host-pre-transposed xT [C, N]:
       q01/k01  [128, N]: heads 0,1 d-major (h0 at partitions 0:64, h1 at
                64:128) -> natural row-tile pairing for the score matmuls.
       q2d/k2d  [128, N]: head 2 duplicated in both partition halves so its
                score matmuls can be row-tile paired across adjacent k-chunks.
       v        [N, 65] per (k-chunk, head): cols 0:64 = v, col 64 = ones
                (softmax denominator trick).
  2. Scores transposed: S^T[k, q] = k_h^T-chunk.T @ q_h. Heads 0/1 (and for
     head 2, adjacent k-chunks) issue as K=64 matmuls at tile_position
     (0,0)/(64,0) -> they stream concurrently in the PE array.
  3. exp via ScalarE (the kernel's throughput floor: ~96 activations of
     [128,1024]); output fp16 to SBUF.
  4. attn@V with lhsT = [v | 1]: psum rows 0:64 = unnormalized attn_out^T,
     row 64 = denominators.  Normalize: gpsimd partition-broadcast of the
     denominator row, DVE reciprocal_approx_fast, DVE multiply -> fp16.
  5. Output projection LOCALLY (w_proj row-shard, 3 K=64 chunks + bias),
     then a 4-core ReduceScatter(add) per q-window delivers each core its
     final [192, N] output shard.  Window 0's RS overlaps window 1 compute.
"""

import numpy as np

B, N, C, H, HD = 2, 2048, 768, 12, 64
G = 4              # tensor-parallel head groups
HL = H // G        # 3 heads per core
CHL = HL * HD      # 192 local channels
SCALE = HD ** -0.5
NCORES = 8
CT = C // 128      # 6 contraction chunks
FW = 512           # matmul free width (psum bank)
QW = 1024          # q window width
NWIN = N // QW     # 2 windows
KT = N // 128      # 16 k chunks
VW = HD + 1        # v tile cols: 64 v + 1 ones

_CACHE = {}


def _build_nc():
    import concourse.bass as bass
    import concourse.bacc as bacc
    import concourse.tile as tile
    import concourse.mybir as mybir

    F32 = mybir.dt.float32
    F16 = mybir.dt.float16
    AF = mybir.ActivationFunctionType
    RG = [[0, 1, 2, 3], [4, 5, 6, 7]]

    nc = bacc.Bacc(num_devices=NCORES)
    xT_d = nc.declare_dram_parameter("xT", [C, N], F16, isOutput=False)
    wqa_d = nc.declare_dram_parameter("wqa", [C, 256], F16, isOutput=False)
    wka_d = nc.declare_dram_parameter("wka", [C, 256], F16, isOutput=False)
    wv_d = nc.declare_dram_parameter("wv", [C, CHL], F16, isOutput=False)
    wp_d = nc.declare_dram_parameter("wp", [CHL, C], F16, isOutput=False)
    bp_d = nc.declare_dram_parameter("bp", [1, C], F16, isOutput=False)
    out_d = nc.declare_dram_parameter("out", [CHL, N], F16, isOutput=True)

    with tile.TileContext(nc) as tc:
        with tc.tile_pool(name="dram", bufs=1, space="DRAM") as dram:
            rs_ins = [dram.tile([C, QW], F16, name=f"rs_in{w}")
                      for w in range(NWIN)]
            rs_outs = [dram.tile([CHL, QW], F16, name=f"rs_out{w}")
                       for w in range(NWIN)]

            with tc.tile_pool(name="sb", bufs=1) as P, \
                    tc.tile_pool(name="ps", bufs=1, space="PSUM") as PS:
                # ---- input DMAs (weights first: small, unblock matmuls) ----
                wka_sb = P.tile([128, CT, 256], F16)
                wqa_sb = P.tile([128, CT, 256], F16)
                for ct in range(CT):
                    nc.sync.dma_start(out=wka_sb[:, ct, :],
                                      in_=wka_d[ct * 128:(ct + 1) * 128, :])
                xT_sb = P.tile([128, CT, N], F16)
                for ct in range(CT):
                    nc.sync.dma_start(out=xT_sb[:, ct, :],
                                      in_=xT_d[ct * 128:(ct + 1) * 128, :])
                for ct in range(CT):
                    nc.sync.dma_start(out=wqa_sb[:, ct, :],
                                      in_=wqa_d[ct * 128:(ct + 1) * 128, :])
                wv_sb = P.tile([128, CT, CHL], F16)
                for ct in range(CT):
                    nc.sync.dma_start(out=wv_sb[:, ct, :],
                                      in_=wv_d[ct * 128:(ct + 1) * 128, :])
                # w_proj row-shard, one [64, C] tile per local head
                wp_sb = P.tile([64, HL, C], F16)
                for h in range(HL):
                    nc.sync.dma_start(out=wp_sb[:, h, :],
                                      in_=wp_d[h * 64:(h + 1) * 64, :])
                bp_sb = P.tile([1, C], F16)
                nc.sync.dma_start(out=bp_sb[:], in_=bp_d[:, :])
                ones_q = P.tile([1, FW], F16)
                nc.vector.memset(ones_q[:], 1.0)

                # ---- persistent QKV results ----
                k01_sb = P.tile([128, N], F16)
                q01_sb = P.tile([128, N], F16)
                k2d_sb = P.tile([128, N], F16)
                q2d_sb = P.tile([128, N], F16)
                # [n, kt, h, VW]: cols 0:64 = v, col 64 = ones
                v_sb = P.tile([128, KT, HL, VW], F16)
                nc.vector.memset(v_sb[:, :, :, HD:VW], 1.0)

                # ---- QKV projections ----
                # psum ring "sc": [128,1024] slots (2 banks) x2 -> also used
                # by score tiles and proj psum later.
                # psum ring "ac": [65,1024] slots (2 banks) x2 -> v_ps and
                # attention accumulators.
                for dst, wsb, mlo in (
                    (k2d_sb, wka_sb, 128),
                    (q2d_sb, wqa_sb, 128),
                    (k01_sb, wka_sb, 0),
                    (q01_sb, wqa_sb, 0),
                ):
                    for f in range(N // FW):
                        qk_ps = PS.tile([128, FW], F32, tag="sc", bufs=2,
                                        padded_shape=[128, QW])
                        for ct in range(CT):
                            nc.tensor.matmul(
                                qk_ps[:],
                                lhsT=wsb[:, ct, mlo:mlo + 128],
                                rhs=xT_sb[:, ct, f * FW:(f + 1) * FW],
                                start=(ct == 0), stop=(ct == CT - 1),
                            )
                        nc.vector.tensor_copy(
                            dst[:, f * FW:(f + 1) * FW], qk_ps[:])
                for nt in range(KT):
                    v_ps = PS.tile([128, CHL], F32, tag="ac", bufs=2,
                                   padded_shape=[128, QW])
                    for ct in range(CT):
                        nc.tensor.matmul(
                            v_ps[:],
                            lhsT=xT_sb[:, ct, nt * 128:(nt + 1) * 128],
                            rhs=wv_sb[:, ct, :],
                            start=(ct == 0), stop=(ct == CT - 1),
                        )
                    nc.vector.tensor_copy(
                        v_sb[:, nt, :, 0:HD],
                        v_ps[:].rearrange("p (h d) -> p h d", h=HL))

                # ---- attention + local proj + per-window ReduceScatter ----
                with tc.tile_pool(name="att_sb", bufs=1) as AS:
                    def scores_pair(w, lhs_tile, rhs_tile, kc0, kc1, Sa, Sb):
                        """Two K=64 score matmuls row-tiled (0,0)/(64,0)."""
                        q0 = w * QW
                        for j in range(QW // FW):
                            js = slice(q0 + j * FW, q0 + (j + 1) * FW)
                            ps_js = slice(j * FW, (j + 1) * FW)
                            nc.tensor.matmul(
                                Sa[:, ps_js],
                                lhsT=lhs_tile[0:64, kc0 * 128:(kc0 + 1) * 128],
                                rhs=rhs_tile[0:64, js],
                            )
                            nc.tensor.matmul(
                                Sb[:, ps_js],
                                lhsT=lhs_tile[64:128, kc1 * 128:(kc1 + 1) * 128],
                                rhs=rhs_tile[64:128, js],
                            )

                    def av_accum(A, E, kc, h, first, last):
                        for j in range(QW // FW):
                            ps_js = slice(j * FW, (j + 1) * FW)
                            nc.tensor.matmul(
                                A[:, ps_js],
                                lhsT=v_sb[:, kc, h, :],
                                rhs=E[:, ps_js],
                                start=first, stop=last,
                            )

                    def normalize(A, at):
                        """at[0:64] = A[0:64] / A[64] (denominator row)."""
                        den = AS.tile([1, QW], F32, tag="den", bufs=2)
                        bcs = AS.tile([64, QW], F32, tag="bcs", bufs=2)
                        rcp = AS.tile([64, QW], F32, tag="rcp", bufs=2)
                        nc.vector.tensor_copy(den[:], A[64:65, :])
                        nc.gpsimd.partition_broadcast(bcs[:], den[:])
                        nc.vector.reciprocal_approx_fast(rcp[:], bcs[:])
                        nc.vector.tensor_mul(at[:], A[0:64, :], rcp[:])

                    def proj_chunk(w, ats, m):
                        """out^T[m-chunk, w] partial = sum_h wp_h.T@at_h + b."""
                        pr = PS.tile([128, QW], F32, tag="sc", bufs=2)
                        ms = slice(m * 128, (m + 1) * 128)
                        for j in range(QW // FW):
                            ps_js = slice(j * FW, (j + 1) * FW)
                            for h in range(HL):
                                nc.tensor.matmul(
                                    pr[:, ps_js],
                                    lhsT=wp_sb[:, h, ms],
                                    rhs=ats[h][:, ps_js],
                                    start=(h == 0), stop=False,
                                )
                            nc.tensor.matmul(
                                pr[:, ps_js],
                                lhsT=bp_sb[:, ms],
                                rhs=ones_q[:],
                                start=False, stop=True,
                            )
                        po = AS.tile([128, QW], F16, tag="po", bufs=3)
                        nc.vector.tensor_copy(po[:], pr[:])
                        nc.sync.dma_start(out=rs_ins[w][ms, :], in_=po[:])

                    def reduce_scatter(ww):
                        nc.gpsimd.collective_compute(
                            "ReduceScatter",
                            mybir.AluOpType.add,
                            replica_groups=RG,
                            ins=[rs_ins[ww].opt()],
                            outs=[rs_outs[ww].opt()],
                        )
                        nc.sync.dma_start(
                            out=out_d[:, ww * QW:(ww + 1) * QW],
                            in_=rs_outs[ww][:, :],
                        )

                    def attn_h2(w, interleave):
                        """Head 2, adjacent-k-chunk row-tile-paired."""
                        A2 = PS.tile([VW, QW], F32, tag="ac", bufs=2,
                                     padded_shape=[128, QW])
                        for kcp in range(KT // 2):
                            kc0, kc1 = 2 * kcp, 2 * kcp + 1
                            Se = PS.tile([128, QW], F32, tag="sc", bufs=2)
                            So = PS.tile([128, QW], F32, tag="sc", bufs=2)
                            scores_pair(w, k2d_sb, q2d_sb, kc0, kc1, Se, So)
                            Ee = AS.tile([128, QW], F16, tag="E", bufs=4)
                            Eo = AS.tile([128, QW], F16, tag="E", bufs=4)
                            nc.scalar.activation(Ee[:], Se[:], AF.Exp,
                                                 scale=SCALE)
                            nc.scalar.activation(Eo[:], So[:], AF.Exp,
                                                 scale=SCALE)
                            av_accum(A2, Ee, kc0, 2, kc0 == 0, False)
                            av_accum(A2, Eo, kc1, 2, False, kc1 == KT - 1)
                            interleave(kcp)
                        return A2

                    def attn_h01(w):
                        """Heads 0/1, head-row-tile-paired."""
                        A0 = PS.tile([VW, QW], F32, tag="ac", bufs=2,
                                     padded_shape=[128, QW])
                        A1 = PS.tile([VW, QW], F32, tag="ac", bufs=2,
                                     padded_shape=[128, QW])
                        for kc in range(KT):
                            S0 = PS.tile([128, QW], F32, tag="sc", bufs=2)
                            S1 = PS.tile([128, QW], F32, tag="sc", bufs=2)
                            scores_pair(w, k01_sb, q01_sb, kc, kc, S0, S1)
                            E0 = AS.tile([128, QW], F16, tag="E", bufs=4)
                            E1 = AS.tile([128, QW], F16, tag="E", bufs=4)
                            nc.scalar.activation(E0[:], S0[:], AF.Exp,
                                                 scale=SCALE)
                            nc.scalar.activation(E1[:], S1[:], AF.Exp,
                                                 scale=SCALE)
                            av_accum(A0, E0, kc, 0, kc == 0, kc == KT - 1)
                            av_accum(A1, E1, kc, 1, kc == 0, kc == KT - 1)
                        return A0, A1

                    # ---- window 0 ----
                    ats0 = [AS.tile([64, QW], F16, tag=f"at{h}", bufs=2,
                                    name=f"at{h}_w0")
                            for h in range(HL)]
                    A2 = attn_h2(0, lambda kcp: None)
                    normalize(A2, ats0[2])
                    A0, A1 = attn_h01(0)
                    normalize(A0, ats0[0])
                    normalize(A1, ats0[1])

                    # ---- window 1 (window-0 proj + RS interleaved) ----
                    ats1 = [AS.tile([64, QW], F16, tag=f"at{h}", bufs=2,
                                    name=f"at{h}_w1")
                            for h in range(HL)]

                    def w1_interleave(kcp):
                        # two proj-w0 m-chunks per odd kcp; RS after the last
                        if kcp in (1, 3, 5):
                            proj_chunk(0, ats0, kcp - 1)
                            proj_chunk(0, ats0, kcp)
                        elif kcp == 6:
                            reduce_scatter(0)

                    A2 = attn_h2(1, w1_interleave)
                    normalize(A2, ats1[2])
                    A0, A1 = attn_h01(1)
                    normalize(A0, ats1[0])
                    normalize(A1, ats1[1])

                    for m in range(C // 128):
                        proj_chunk(1, ats1, m)
                    reduce_scatter(1)
    nc.finalize()
    return nc


def get_nc():
    if "nc" not in _CACHE:
        _CACHE["nc"] = _build_nc()
    return _CACHE["nc"]


def make_in_maps(x, w_qkv, w_proj, b_proj):
    x = np.asarray(x, dtype=np.float32)
    w_qkv = np.asarray(w_qkv, dtype=np.float32)
    w_proj = np.asarray(w_proj, dtype=np.float32)
    b_proj = np.asarray(b_proj, dtype=np.float32)
    in_maps = []
    for core in range(NCORES):
        b, g = divmod(core, G)
        cs = slice(g * CHL, (g + 1) * CHL)
        wq = w_qkv[:, 0 * C:1 * C][:, cs]
        wk = w_qkv[:, 1 * C:2 * C][:, cs]
        wv = w_qkv[:, 2 * C:3 * C][:, cs]
        # [heads01 | head2 | head2-dup]
        wqa = np.concatenate([wq[:, 0:128], wq[:, 128:192], wq[:, 128:192]],
                             axis=1)
        wka = np.concatenate([wk[:, 0:128], wk[:, 128:192], wk[:, 128:192]],
                             axis=1)
        bp = b_proj if g == 0 else np.zeros_like(b_proj)
        im = {
            "xT": np.ascontiguousarray(x[b].T, dtype=np.float16),
            "wqa": np.ascontiguousarray(wqa, dtype=np.float16),
            "wka": np.ascontiguousarray(wka, dtype=np.float16),
            "wv": np.ascontiguousarray(wv, dtype=np.float16),
            "wp": np.ascontiguousarray(w_proj[cs, :], dtype=np.float16),
            "bp": np.ascontiguousarray(bp.reshape(1, C), dtype=np.float16),
        }
        in_maps.append(im)
    return in_maps


def unshard(results):
    out = np.empty((B, N, C), dtype=np.float32)
    for b in range(B):
        outT = np.concatenate(
            [np.asarray(results[b * G + g]["out"], dtype=np.float32)
             for g in range(G)], axis=0)
        out[b] = outT.T
    return out


def kernel(x, w_qkv, w_proj, b_proj):
    from concourse.bass_utils import run_bass_kernel_spmd

    nc = get_nc()
    in_maps = make_in_maps(x, w_qkv, w_proj, b_proj)
    res = run_bass_kernel_spmd(nc, in_maps, list(range(NCORES)))
    return unshard(res.results)
